# revision 15
# baseline (speedup 1.0000x reference)
"""CoAttention kernel for Trainium2, data-parallel over batch across 8 NeuronCores.

Per core (one batch element b):
    query = data1[b] @ Wq + bq                      # [2048, 256]
    key   = data2[b] @ Wk + bk                      # [2048, 256]
    attn  = softmax(SCALE * query @ key^T)          # row-constant terms cancel
    out   = attn @ key + query

Device-side strategy (v4):
  - All input DMAs issue up-front: activations on the sync HWDGE ring
    (d1g0, d2h0, d1g1, d2h1, d1g2, d1g3), weights + biases on the scalar
    HWDGE ring in parallel.  fp32 -> bf16 casts run on DVE (activations)
    and ACT (weights, idle pre-exp).
  - softmax(q@(k+bk)^T) drops bias terms constant along k, and
    sum(attn)==1 makes attn@(key+bk) == attn@key + bk, so the key value
    matrix carries NO bias; bq biases the scores path and (bq+bk) the
    residual path from the same QT PSUM.  The softmax denominator is a
    memset 1.0 column appended to the fp8 key values.
  - Transposes: early ones (d1 g0/g1, d2 h0) run on the PE while it is
    otherwise idle; mid-kernel ones (d1 g2/g3, d2 h1, Q-residual) run as
    batched xbar DMA transposes ([128, n*128] -> [128, n, 128]) on the
    sync ring, freeing the PE during the exp-bound phase.
  - scoresT [k, q] orientation lets exp(scoresT) feed the context matmul
    as the stationary operand; scores and context run in fp8e4m3
    DoubleRow.  ctx for the second q-half is split kp0-5 (runs inside the
    exp stream, evicted to bf16) + kp6-7 (after the last exp) to shrink
    the serial tail.
  - ACT does only the 32 serial exps plus small pre-exp work; GPSIMD never
    touches PSUM.  24 warmup matmuls ramp the PE p-state early.
  - Output is written in 8 chunks of 256 rows as each completes.
"""

import sys

if "/opt/trn_rl_repo" not in sys.path:
    sys.path.insert(0, "/opt/trn_rl_repo")

from contextlib import ExitStack

import numpy as np

import concourse.bass as bass  # noqa: F401
import concourse.mybir as mybir
import concourse.tile as tile
from concourse import bacc
from concourse.bass_utils import run_bass_kernel_spmd
from concourse.masks import make_identity

B, LQ, LK, DIN, D = 8, 2048, 2048, 1024, 256
N_CORES = 8
SCALE = float(1.0 / np.sqrt(1024.0).astype(np.float32))

BF16 = mybir.dt.bfloat16
FP8 = mybir.dt.float8e4
F32 = mybir.dt.float32
AF = mybir.ActivationFunctionType
PM_DR = mybir.MatmulPerfMode.DoubleRow
ADD = mybir.AluOpType.add
MULT = mybir.AluOpType.mult


def _build():
    nc = bacc.Bacc("TRN2", target_bir_lowering=False, debug=False)
    d1 = nc.dram_tensor("data1", [LQ, DIN], F32, kind="ExternalInput").ap()
    d2 = nc.dram_tensor("data2", [LK, D], F32, kind="ExternalInput").ap()
    wpk = nc.dram_tensor("wpack", [128, 2564], F32, kind="ExternalInput").ap()
    out = nc.dram_tensor("out", [LQ, D], F32, kind="ExternalOutput").ap()

    WPACK = 2564
    QB = LQ // 128  # 16 q blocks
    KB = LK // 128  # 16 k blocks
    IC1 = DIN // 128  # 8
    IC2 = D // 128  # 2
    KP = KB // 2  # 8 fp8 DoubleRow k-pairs

    with tile.TileContext(nc) as tc, ExitStack() as ctx:
        const = ctx.enter_context(tc.tile_pool(name="const", bufs=1))
        big = ctx.enter_context(tc.tile_pool(name="big", bufs=1))
        stage1 = ctx.enter_context(tc.tile_pool(name="stage1", bufs=3))
        stage2 = ctx.enter_context(tc.tile_pool(name="stage2", bufs=2))
        small = ctx.enter_context(tc.tile_pool(name="small", bufs=4))
        ps_gp = ctx.enter_context(tc.tile_pool(name="ps_gp", bufs=2, space="PSUM"))
        ps_tr = ctx.enter_context(tc.tile_pool(name="ps_tr", bufs=2, space="PSUM"))
        ps_sc = ctx.enter_context(tc.tile_pool(name="ps_sc", bufs=2, space="PSUM"))

        # ---------------- constants ----------------
        ident_bf = const.tile([128, 128], BF16, tag="ident_bf")
        make_identity(nc, ident_bf[:])
        warm_src = const.tile([128, 512], BF16, tag="warm_src")
        nc.gpsimd.memset(warm_src[:], 0.0)

        key2 = [
            big.tile([128, 2, D + 1], FP8, tag=f"key2_{kp}", name=f"key2_{kp}")
            for kp in range(KP)
        ]
        for kp in range(KP):
            nc.gpsimd.memset(key2[kp][:, :, D:D + 1], 1.0)

        # ---------------- loads -------------------------------------------
        # wpack = host-packed [wq | wk | bq | bk] in final SBUF layout: one
        # fast contiguous DMA first on the sync ring, then the d1 groups.
        # d2 halves ride the scalar ring (only 2 issue ops on the ACT queue).
        d2_st = [stage2.tile([128, 8 * D], F32, tag="d2st", name=f"d2st{hh}")
                 for hh in range(2)]
        d1_st = [stage1.tile([128, 4 * DIN], F32, tag="d1st", name=f"d1st{g}")
                 for g in range(4)]
        wpack_st = const.tile([128, WPACK], F32, tag="wpack")

        def load_d2(hh):
            nc.sync.dma_start(
                out=d2_st[hh][:].rearrange("p (t i) -> p t i", i=D),
                in_=d2[hh * 1024:(hh + 1) * 1024, :].rearrange(
                    "(t p) i -> p t i", p=128),
            )

        def load_d1(g):
            nc.sync.dma_start(
                out=d1_st[g][:].rearrange("p (t i) -> p t i", i=DIN),
                in_=d1[g * 512:(g + 1) * 512, :].rearrange("(t p) i -> p t i", p=128),
            )

        load_d1(0)
        nc.sync.dma_start(out=wpack_st[:], in_=wpk)
        load_d1(1)
        load_d2(0)
        load_d2(1)
        load_d1(2)
        load_d1(3)
        bq_col = wpack_st[:, 2560:2562]
        bk_col = wpack_st[:, 2562:2564]

        # ---------------- PE p-state warmup ---------------------------------
        for w in range(12):
            pw = ps_gp.tile([128, 512], F32, tag="ps_gp", name=f"warm{w}")
            nc.tensor.matmul(pw[:], lhsT=ident_bf[:], rhs=warm_src[:],
                             start=True, stop=True)

        # ---------------- weight cast (ACT) + residual bias (gpsimd) --------
        wqk_sb = const.tile([128, IC1 * D + IC2 * D], BF16, tag="wqk_sb")
        nc.scalar.copy(wqk_sb[:], wpack_st[:, :2560])
        wqs = [wqk_sb[:, i * D:(i + 1) * D] for i in range(IC1)]
        wks = [wqk_sb[:, 2048 + i * D:2048 + (i + 1) * D] for i in range(IC2)]
        bqk_col = const.tile([128, IC2], F32, tag="bqk_col")
        nc.gpsimd.tensor_add(bqk_col[:], bq_col[:], bk_col[:])

        # ---------------- transposed activations ----------------------------
        # d1T[:, ic, q] : d1[q, ic*128 + p];  d2T[:, ic, k] : d2[k, ic*128 + p]
        d1T = big.tile([128, IC1, LQ], BF16, tag="d1T")
        d2T = big.tile([128, IC2, LK], BF16, tag="d2T")
        d1_bf = [stage2.tile([128, 4 * DIN], BF16, tag="d1bf", name=f"d1bf{g}",
                             bufs=1)
                 for g in range(4)]
        d2_bf = [stage2.tile([128, 8 * D], BF16, tag="d2bf", name=f"d2bf{hh}",
                             bufs=1)
                 for hh in range(2)]

        def d1_cast(g, on_act=False):
            if on_act:
                nc.scalar.copy(d1_bf[g][:], d1_st[g][:])
            else:
                nc.vector.tensor_copy(d1_bf[g][:], d1_st[g][:])

        def d1_cast_ic(g, ic):
            src = d1_st[g][:].rearrange("p (t i) -> p t i", i=DIN)
            dst = d1_bf[g][:].rearrange("p (t i) -> p t i", i=DIN)
            nc.vector.tensor_copy(
                dst[:, :, ic * 128:(ic + 1) * 128],
                src[:, :, ic * 128:(ic + 1) * 128],
            )

        def d2_cast(hh):
            nc.vector.tensor_copy(d2_bf[hh][:], d2_st[hh][:])

        # PE transpose path (early groups, PE otherwise idle)
        def d1T_pe(g, ic):
            bf = d1_bf[g]
            pt = ps_tr.tile([128, 512], BF16, tag="ps_t", name=f"pt_d1_{g}_{ic}")
            for j in range(4):
                nc.tensor.transpose(
                    pt[:, j * 128:(j + 1) * 128],
                    bf[:, j * DIN + ic * 128: j * DIN + (ic + 1) * 128],
                    ident_bf[:],
                )
            nc.vector.tensor_copy(d1T[:, ic, g * 512:(g + 1) * 512], pt[:])

        def d2T_pe(hh, ic, h4):
            bf = d2_bf[hh]
            pt = ps_tr.tile([128, 512], BF16, tag="ps_t",
                            name=f"pt_d2_{hh}_{ic}_{h4}")
            for j in range(4):
                kt = 4 * h4 + j
                nc.tensor.transpose(
                    pt[:, j * 128:(j + 1) * 128],
                    bf[:, kt * D + ic * 128: kt * D + (ic + 1) * 128],
                    ident_bf[:],
                )
            nc.vector.tensor_copy(
                d2T[:, ic, hh * 1024 + h4 * 512: hh * 1024 + (h4 + 1) * 512],
                pt[:],
            )


        # ---------------- K^T fp8 DoubleRow layout [128, 2, k] --------------
        kt_sb = big.tile([128, 2, LK], FP8, tag="kt_sb")

        def kt_unit(dc, nk):
            ps = ps_gp.tile([128, 512], F32, tag="ps_gp")
            for ic in range(IC2):
                nc.tensor.matmul(
                    ps[:],
                    lhsT=wks[ic][:, dc * 128:(dc + 1) * 128],
                    rhs=d2T[:, ic, nk * 512:(nk + 1) * 512],
                    start=(ic == 0),
                    stop=(ic == IC2 - 1),
                )
            nc.vector.tensor_copy(kt_sb[:, dc, nk * 512:(nk + 1) * 512], ps[:])

        # ---------------- key value pairs (no bias) --------------------------
        def key_pair(kp):
            ps = ps_gp.tile([128, 512], F32, tag="ps_gp")
            for s in range(2):
                kb = kp * 2 + s
                p = ps[:, s * D:(s + 1) * D]
                for ic in range(IC2):
                    nc.tensor.matmul(
                        p,
                        lhsT=d2T[:, ic, kb * 128:(kb + 1) * 128],
                        rhs=wks[ic],
                        start=(ic == 0),
                        stop=(ic == IC2 - 1),
                    )
            nc.vector.tensor_copy(
                key2[kp][:, :, :D],
                ps[:].rearrange("p (s d) -> p s d", s=2),
            )

        # ---------------- QT projection ------------------------------------
        qt_sb = big.tile([128, 2, LQ], FP8, tag="qt_sb")
        qtbf = big.tile([128, 2, LQ], BF16, tag="qtbf")

        def qt_mms(dc, nq, ps):
            for ic in range(IC1):
                nc.tensor.matmul(
                    ps[:],
                    lhsT=wqs[ic][:, dc * 128:(dc + 1) * 128],
                    rhs=d1T[:, ic, nq * 512:(nq + 1) * 512],
                    start=(ic == 0),
                    stop=(ic == IC1 - 1),
                )

        def qt_bias_sc(ps, dc, nq, on_act):
            o = qt_sb[:, dc, nq * 512:(nq + 1) * 512]
            if on_act:
                nc.scalar.activation(o, ps[:], AF.Identity,
                                     bias=bq_col[:, dc:dc + 1])
            else:
                nc.vector.tensor_scalar(o, ps[:], bq_col[:, dc:dc + 1], None, ADD)

        def qt_bias_rs(ps, dc, nq, on_act):
            o = qtbf[:, dc, nq * 512:(nq + 1) * 512]
            if on_act:
                nc.scalar.activation(o, ps[:], AF.Identity,
                                     bias=bqk_col[:, dc:dc + 1])
            else:
                nc.vector.tensor_scalar(o, ps[:], bqk_col[:, dc:dc + 1], None, ADD)

        def qt_unit(dc, nq, on_act=False):
            ps = ps_gp.tile([128, 512], F32, tag="ps_gp")
            qt_mms(dc, nq, ps)
            qt_bias_sc(ps, dc, nq, on_act)
            qt_bias_rs(ps, dc, nq, on_act)

        # ---------------- residual Q via xbar DMA transpose ------------------
        # qres3[qg][q_low, j, dc, c] = Q[qg*512 + j*128 + q_low, dc*128 + c]
        qres3 = [big.tile([128, 4, 2, 128], BF16, tag=f"qres{qg}",
                          name=f"qres{qg}")
                 for qg in range(4)]

        def qres_pe(qg, dc):
            pt = ps_tr.tile([128, 512], BF16, tag="ps_t", name=f"pt_q_{qg}_{dc}")
            for j in range(4):
                qb = qg * 4 + j
                nc.tensor.transpose(
                    pt[:, j * 128:(j + 1) * 128],
                    qtbf[:, dc, qb * 128:(qb + 1) * 128],
                    ident_bf[:],
                )
            nc.vector.tensor_copy(
                qres3[qg][:, :, dc, :],
                pt[:].rearrange("p (j c) -> p j c", c=128),
            )

        # ---------------- scores + exp --------------------------------------
        expT = [
            [big.tile([128, 2, 1024], FP8, tag=f"expT{kp}_{nh}",
                      name=f"expT{kp}_{nh}")
             for nh in range(2)]
            for kp in range(KP)
        ]

        def scores_unit(km, nh):
            ps = ps_sc.tile([128, 1024], F32, tag="ps_sc")
            for half in range(2):
                nq = nh * 2 + half
                nc.tensor.matmul(
                    ps[:, half * 512:(half + 1) * 512],
                    lhsT=kt_sb[:, :, km * 128:(km + 1) * 128],
                    rhs=qt_sb[:, :, nq * 512:(nq + 1) * 512],
                    perf_mode=PM_DR,
                    start=True,
                    stop=True,
                )
            nc.scalar.activation(
                expT[km // 2][nh][:, km % 2, :], ps[:], AF.Exp, scale=SCALE
            )

        # ---------------- context + residual + out DMA ----------------------
        out_c = [stage2.tile([128, 2 * D], F32, tag="outc", name=f"outc{c}")
                 for c in range(QB // 2)]
        ctxA = [big.tile([128, D + 1], BF16, tag=f"ctxA{i}", name=f"ctxA{i}")
                for i in range(8)]
        KP_A = 6  # h1 ctx kp-split: A = kp0-5 inside exp stream, B = kp6-7 after

        def ctx_mm(pc, qb, kp, start, stop):
            h, qq = qb // 8, qb % 8
            nc.tensor.matmul(
                pc,
                lhsT=expT[kp][h][:, :, qq * 128:(qq + 1) * 128],
                rhs=key2[kp][:],
                perf_mode=PM_DR,
                start=start,
                stop=stop,
            )

        def ctx_finish(pc, qb):
            rc = small.tile([128, 1], F32, tag="recip")
            nc.vector.reciprocal(rc[:], pc[:, D:D + 1])
            c = qb // 2
            osl = out_c[c][:, (qb % 2) * D:(qb % 2 + 1) * D]
            nc.vector.tensor_scalar(osl, pc[:, :D], rc[:], None, MULT)
            qg, j = qb // 4, qb % 4
            nc.gpsimd.tensor_add(
                osl.rearrange("p (a b) -> p a b", a=2),
                osl.rearrange("p (a b) -> p a b", a=2),
                qres3[qg][:, j, :, :],
            )
            if qb % 2 == 1:
                nc.sync.dma_start(
                    out=out[c * 256:(c + 1) * 256, :].rearrange(
                        "(t p) d -> p t d", p=128),
                    in_=out_c[c][:].rearrange("p (t d) -> p t d", d=D),
                )

        def ctx_unit_h0(qb):
            pc_full = ps_gp.tile([128, 512], F32, tag="ps_gp")
            pc = pc_full[:, :D + 1]
            for kp in range(KP):
                ctx_mm(pc, qb, kp, kp == 0, kp == KP - 1)
            ctx_finish(pc, qb)

        def ctx_h1_A(qb):
            pc_full = ps_gp.tile([128, 512], F32, tag="ps_gp")
            pc = pc_full[:, :D + 1]
            for kp in range(KP_A):
                ctx_mm(pc, qb, kp, kp == 0, kp == KP_A - 1)
            nc.vector.tensor_copy(ctxA[qb - 8][:], pc)

        def ctx_h1_B(qb):
            pc_full = ps_sc.tile([128, 512], F32, tag="ps_sc")
            pc = pc_full[:, :D + 1]
            for kp in range(KP_A, KP):
                ctx_mm(pc, qb, kp, kp == KP_A, kp == KP - 1)
            nc.vector.tensor_tensor(pc, pc, ctxA[qb - 8][:], ADD)
            ctx_finish(pc, qb)

        # ================= emission schedule ================================
        def units(fn, idxs):
            return [lambda i=i: fn(*i) if isinstance(i, tuple) else fn(i)
                    for i in idxs]

        def interleave(a, b, ratio):
            a = list(a)
            b = list(b)
            ia = ib = 0
            credit = 0.0
            while ia < len(a) or ib < len(b):
                if ia < len(a):
                    a[ia]()
                    ia += 1
                credit += ratio
                while credit >= 1.0 and ib < len(b):
                    b[ib]()
                    ib += 1
                    credit -= 1.0
            while ib < len(b):
                b[ib]()
                ib += 1

        # --- phase 1a: d1 g0 -> PE transposes -> QT nq0 (d1 lands first) ---
        d1_cast(0, on_act=True)
        for ic in range(IC1):
            d1T_pe(0, ic)
        for dc in range(2):
            qt_unit(dc, 0, on_act=True)

        # --- phase 1b: d2 h0 -> PE transposes -> kt-h0 ---
        d2_cast(0)
        for ic in range(IC2):
            for h4 in range(2):
                d2T_pe(0, ic, h4)
        for nk in range(2):
            for dc in range(2):
                kt_unit(dc, nk)

        # --- phase 1c: d1 g1 per-ic pipelined -> QT nq1 (gates exp#0) ---
        ps_nq1 = []
        for dc in range(2):
            ps = ps_gp.tile([128, 512], F32, tag="ps_gp", name=f"qtps1_{dc}")
            for ic in range(IC1):
                if dc == 0:
                    d1_cast_ic(1, ic)
                    d1T_pe(1, ic)
                nc.tensor.matmul(
                    ps[:],
                    lhsT=wqs[ic][:, dc * 128:(dc + 1) * 128],
                    rhs=d1T[:, ic, 512:1024],
                    start=(ic == 0),
                    stop=(ic == IC1 - 1),
                )
            ps_nq1.append(ps)
        for dc in range(2):
            qt_bias_sc(ps_nq1[dc], dc, 1, on_act=True)
        for dc in range(2):
            qt_bias_rs(ps_nq1[dc], dc, 1, on_act=False)

        # residual transposes for q-half 0 (PE)
        for qg in range(2):
            for dc in range(2):
                qres_pe(qg, dc)

        # --- phase 2: scores-h0 interleaved with late loads + DMA T ---
        filler = (
            [lambda: d2_cast(1)]
            + units(d2T_pe, [(1, ic, h4) for ic in range(IC2)
                             for h4 in range(2)])
            + units(kt_unit, [(dc, nk) for nk in range(2, 4)
                              for dc in range(2)])
            + [lambda: d1_cast(2)]
            + units(d1T_pe, [(2, ic) for ic in range(IC1)])
            + units(qt_unit, [(0, 2), (1, 2)])
            + [lambda: d1_cast(3)]
            + units(d1T_pe, [(3, ic) for ic in range(IC1)])
            + units(qt_unit, [(0, 3), (1, 3)])
            + units(qres_pe, [(qg, dc) for qg in range(2, 4)
                              for dc in range(2)])
            + units(key_pair, list(range(8)))
        )
        scores_h0 = units(scores_unit, [(km, 0) for km in range(KB)])
        interleave(scores_h0, filler, len(filler) / len(scores_h0))

        # --- phase 3: scores-h1 with ctx-h0 and ctx-h1-A ---
        # ctx_h1_A reads exps km0-11 of h1, so A units may only be emitted
        # after scores-h1 km11 (Tile orders by emission-time dependencies)
        sc_h1_a = units(scores_unit, [(km, 1) for km in range(12)])
        sc_h1_b = units(scores_unit, [(km, 1) for km in range(12, KB)])
        ctx0 = units(ctx_unit_h0, list(range(0, 8)))
        ctxa = units(ctx_h1_A, list(range(8, 16)))
        interleave(sc_h1_a, ctx0, len(ctx0) / len(sc_h1_a))
        interleave(sc_h1_b, ctxa, len(ctxa) / len(sc_h1_b))

        # --- phase 4: ctx-h1-B tail ---
        for qb in range(8, 16):
            ctx_h1_B(qb)

    nc.compile()
    return nc


_NC = None
_last_in_maps = None


def _get_nc():
    global _NC
    if _NC is None:
        _NC = _build()
    return _NC


def make_wpack(Wq, Wk, bq, bk):
    Wq = np.asarray(Wq, dtype=np.float32)
    Wk = np.asarray(Wk, dtype=np.float32)
    bq = np.asarray(bq, dtype=np.float32)
    bk = np.asarray(bk, dtype=np.float32)
    wpack = np.empty((128, 2564), np.float32)
    for c in range(8):
        wpack[:, c * 256:(c + 1) * 256] = Wq[c * 128:(c + 1) * 128, :]
    for c in range(2):
        wpack[:, 2048 + c * 256:2048 + (c + 1) * 256] = Wk[c * 128:(c + 1) * 128, :]
    for c in range(2):
        wpack[:, 2560 + c] = bq[c * 128:(c + 1) * 128]
        wpack[:, 2562 + c] = bk[c * 128:(c + 1) * 128]
    return np.ascontiguousarray(wpack)


def kernel(data1, data2, Wq, bq, Wk, bk):
    global _last_in_maps
    data1 = np.asarray(data1, dtype=np.float32)
    data2 = np.asarray(data2, dtype=np.float32)
    wpack = make_wpack(Wq, Wk, bq, bk)

    nc = _get_nc()
    in_maps = [
        {
            "data1": np.ascontiguousarray(data1[b]),
            "data2": np.ascontiguousarray(data2[b]),
            "wpack": wpack,
        }
        for b in range(B)
    ]
    _last_in_maps = in_maps
    res = run_bass_kernel_spmd(nc, in_maps, core_ids=list(range(N_CORES)))
    return np.stack([res.results[i]["out"] for i in range(B)], axis=0)


# revision 16
# speedup vs baseline: 1.0285x; 1.0285x over previous
"""CoAttention kernel for Trainium2, data-parallel over batch across 8 NeuronCores.

Per core (one batch element b):
    query = data1[b] @ Wq + bq                      # [2048, 256]
    key   = data2[b] @ Wk + bk                      # [2048, 256]
    attn  = softmax(SCALE * query @ key^T)          # row-constant terms cancel
    out   = attn @ key + query

Device-side strategy (v4):
  - All input DMAs issue up-front: activations on the sync HWDGE ring
    (d1g0, d2h0, d1g1, d2h1, d1g2, d1g3), weights + biases on the scalar
    HWDGE ring in parallel.  fp32 -> bf16 casts run on DVE (activations)
    and ACT (weights, idle pre-exp).
  - softmax(q@(k+bk)^T) drops bias terms constant along k, and
    sum(attn)==1 makes attn@(key+bk) == attn@key + bk, so the key value
    matrix carries NO bias; bq biases the scores path and (bq+bk) the
    residual path from the same QT PSUM.  The softmax denominator is a
    memset 1.0 column appended to the fp8 key values.
  - Transposes: early ones (d1 g0/g1, d2 h0) run on the PE while it is
    otherwise idle; mid-kernel ones (d1 g2/g3, d2 h1, Q-residual) run as
    batched xbar DMA transposes ([128, n*128] -> [128, n, 128]) on the
    sync ring, freeing the PE during the exp-bound phase.
  - scoresT [k, q] orientation lets exp(scoresT) feed the context matmul
    as the stationary operand; scores and context run in fp8e4m3
    DoubleRow.  ctx for the second q-half is split kp0-5 (runs inside the
    exp stream, evicted to bf16) + kp6-7 (after the last exp) to shrink
    the serial tail.
  - ACT does only the 32 serial exps plus small pre-exp work; GPSIMD never
    touches PSUM.  24 warmup matmuls ramp the PE p-state early.
  - Output is written in 8 chunks of 256 rows as each completes.
"""

import sys

if "/opt/trn_rl_repo" not in sys.path:
    sys.path.insert(0, "/opt/trn_rl_repo")

from contextlib import ExitStack

import numpy as np

import concourse.bass as bass  # noqa: F401
import concourse.mybir as mybir
import concourse.tile as tile
from concourse import bacc
from concourse.bass_utils import run_bass_kernel_spmd
from concourse.masks import make_identity

B, LQ, LK, DIN, D = 8, 2048, 2048, 1024, 256
N_CORES = 8
SCALE = float(1.0 / np.sqrt(1024.0).astype(np.float32))

BF16 = mybir.dt.bfloat16
FP8 = mybir.dt.float8e4
F32 = mybir.dt.float32
AF = mybir.ActivationFunctionType
PM_DR = mybir.MatmulPerfMode.DoubleRow
ADD = mybir.AluOpType.add
MULT = mybir.AluOpType.mult


def _build():
    nc = bacc.Bacc("TRN2", target_bir_lowering=False, debug=False)
    d1 = nc.dram_tensor("data1", [LQ, DIN], F32, kind="ExternalInput").ap()
    d2 = nc.dram_tensor("data2", [LK, D], F32, kind="ExternalInput").ap()
    wpk = nc.dram_tensor("wpack", [128, 2564], F32, kind="ExternalInput").ap()
    out = nc.dram_tensor("out", [LQ, D], F32, kind="ExternalOutput").ap()

    WPACK = 2564
    QB = LQ // 128  # 16 q blocks
    KB = LK // 128  # 16 k blocks
    IC1 = DIN // 128  # 8
    IC2 = D // 128  # 2
    KP = KB // 2  # 8 fp8 DoubleRow k-pairs

    with tile.TileContext(nc) as tc, ExitStack() as ctx:
        const = ctx.enter_context(tc.tile_pool(name="const", bufs=1))
        big = ctx.enter_context(tc.tile_pool(name="big", bufs=1))
        stage1 = ctx.enter_context(tc.tile_pool(name="stage1", bufs=3))
        stage2 = ctx.enter_context(tc.tile_pool(name="stage2", bufs=2))
        small = ctx.enter_context(tc.tile_pool(name="small", bufs=4))
        ps_gp = ctx.enter_context(tc.tile_pool(name="ps_gp", bufs=2, space="PSUM"))
        ps_tr = ctx.enter_context(tc.tile_pool(name="ps_tr", bufs=2, space="PSUM"))
        ps_sc = ctx.enter_context(tc.tile_pool(name="ps_sc", bufs=2, space="PSUM"))

        # ---------------- constants ----------------
        ident_bf = const.tile([128, 128], BF16, tag="ident_bf")
        make_identity(nc, ident_bf[:])
        warm_src = const.tile([128, 512], BF16, tag="warm_src")
        nc.gpsimd.memset(warm_src[:], 0.0)

        key2 = [
            big.tile([128, 2, D + 1], FP8, tag=f"key2_{kp}", name=f"key2_{kp}")
            for kp in range(KP)
        ]
        for kp in range(KP):
            nc.gpsimd.memset(key2[kp][:, :, D:D + 1], 1.0)

        # ---------------- loads -------------------------------------------
        # wpack = host-packed [wq | wk | bq | bk] in final SBUF layout: one
        # fast contiguous DMA first on the sync ring, then the d1 groups.
        # d2 halves ride the scalar ring (only 2 issue ops on the ACT queue).
        d2_st = [stage2.tile([128, 8 * D], F32, tag="d2st", name=f"d2st{hh}")
                 for hh in range(2)]
        d1_st = [stage1.tile([128, 4 * DIN], F32, tag="d1st", name=f"d1st{g}")
                 for g in range(4)]
        wpack_st = const.tile([128, WPACK], F32, tag="wpack")

        def load_d2(hh):
            nc.sync.dma_start(
                out=d2_st[hh][:].rearrange("p (t i) -> p t i", i=D),
                in_=d2[hh * 1024:(hh + 1) * 1024, :].rearrange(
                    "(t p) i -> p t i", p=128),
            )

        def load_d1(g):
            nc.sync.dma_start(
                out=d1_st[g][:].rearrange("p (t i) -> p t i", i=DIN),
                in_=d1[g * 512:(g + 1) * 512, :].rearrange("(t p) i -> p t i", p=128),
            )

        nc.sync.dma_start(out=wpack_st[:], in_=wpk)
        load_d1(0)
        load_d1(1)
        load_d2(0)
        load_d2(1)
        load_d1(2)
        load_d1(3)
        bq_col = wpack_st[:, 2560:2562]
        bk_col = wpack_st[:, 2562:2564]

        # ---------------- PE p-state warmup ---------------------------------
        for w in range(12):
            pw = ps_gp.tile([128, 512], F32, tag="ps_gp", name=f"warm{w}")
            nc.tensor.matmul(pw[:], lhsT=ident_bf[:], rhs=warm_src[:],
                             start=True, stop=True)

        # ---------------- weight cast (ACT) + residual bias (gpsimd) --------
        wqk_sb = const.tile([128, IC1 * D + IC2 * D], BF16, tag="wqk_sb")
        nc.scalar.copy(wqk_sb[:], wpack_st[:, :2560])
        wqs = [wqk_sb[:, i * D:(i + 1) * D] for i in range(IC1)]
        wks = [wqk_sb[:, 2048 + i * D:2048 + (i + 1) * D] for i in range(IC2)]
        bqk_col = const.tile([128, IC2], F32, tag="bqk_col")
        nc.gpsimd.tensor_add(bqk_col[:], bq_col[:], bk_col[:])

        # ---------------- transposed activations ----------------------------
        # d1T[:, ic, q] : d1[q, ic*128 + p];  d2T[:, ic, k] : d2[k, ic*128 + p]
        d1T = big.tile([128, IC1, LQ], BF16, tag="d1T")
        d2T = big.tile([128, IC2, LK], BF16, tag="d2T")
        d1_bf = [stage2.tile([128, 4 * DIN], BF16, tag="d1bf", name=f"d1bf{g}",
                             bufs=1)
                 for g in range(4)]
        d2_bf = [stage2.tile([128, 8 * D], BF16, tag="d2bf", name=f"d2bf{hh}",
                             bufs=1)
                 for hh in range(2)]

        def d1_cast(g, on_act=False):
            if on_act:
                nc.scalar.copy(d1_bf[g][:], d1_st[g][:])
            else:
                nc.vector.tensor_copy(d1_bf[g][:], d1_st[g][:])

        def d1_cast_ic(g, ic):
            src = d1_st[g][:].rearrange("p (t i) -> p t i", i=DIN)
            dst = d1_bf[g][:].rearrange("p (t i) -> p t i", i=DIN)
            nc.vector.tensor_copy(
                dst[:, :, ic * 128:(ic + 1) * 128],
                src[:, :, ic * 128:(ic + 1) * 128],
            )

        def d2_cast(hh):
            nc.vector.tensor_copy(d2_bf[hh][:], d2_st[hh][:])

        # PE transpose path (early groups, PE otherwise idle)
        def d1T_pe(g, ic):
            bf = d1_bf[g]
            pt = ps_tr.tile([128, 512], BF16, tag="ps_t", name=f"pt_d1_{g}_{ic}")
            for j in range(4):
                nc.tensor.transpose(
                    pt[:, j * 128:(j + 1) * 128],
                    bf[:, j * DIN + ic * 128: j * DIN + (ic + 1) * 128],
                    ident_bf[:],
                )
            nc.vector.tensor_copy(d1T[:, ic, g * 512:(g + 1) * 512], pt[:])

        def d2T_pe(hh, ic, h4):
            bf = d2_bf[hh]
            pt = ps_tr.tile([128, 512], BF16, tag="ps_t",
                            name=f"pt_d2_{hh}_{ic}_{h4}")
            for j in range(4):
                kt = 4 * h4 + j
                nc.tensor.transpose(
                    pt[:, j * 128:(j + 1) * 128],
                    bf[:, kt * D + ic * 128: kt * D + (ic + 1) * 128],
                    ident_bf[:],
                )
            nc.vector.tensor_copy(
                d2T[:, ic, hh * 1024 + h4 * 512: hh * 1024 + (h4 + 1) * 512],
                pt[:],
            )


        # ---------------- K^T fp8 DoubleRow layout [128, 2, k] --------------
        kt_sb = big.tile([128, 2, LK], FP8, tag="kt_sb")

        def kt_unit(dc, nk):
            ps = ps_gp.tile([128, 512], F32, tag="ps_gp")
            for ic in range(IC2):
                nc.tensor.matmul(
                    ps[:],
                    lhsT=wks[ic][:, dc * 128:(dc + 1) * 128],
                    rhs=d2T[:, ic, nk * 512:(nk + 1) * 512],
                    start=(ic == 0),
                    stop=(ic == IC2 - 1),
                )
            nc.vector.tensor_copy(kt_sb[:, dc, nk * 512:(nk + 1) * 512], ps[:])

        # ---------------- key value pairs (no bias) --------------------------
        def key_pair(kp):
            ps = ps_gp.tile([128, 512], F32, tag="ps_gp")
            for s in range(2):
                kb = kp * 2 + s
                p = ps[:, s * D:(s + 1) * D]
                for ic in range(IC2):
                    nc.tensor.matmul(
                        p,
                        lhsT=d2T[:, ic, kb * 128:(kb + 1) * 128],
                        rhs=wks[ic],
                        start=(ic == 0),
                        stop=(ic == IC2 - 1),
                    )
            nc.vector.tensor_copy(
                key2[kp][:, :, :D],
                ps[:].rearrange("p (s d) -> p s d", s=2),
            )

        # ---------------- QT projection ------------------------------------
        qt_sb = big.tile([128, 2, LQ], FP8, tag="qt_sb")
        qtbf = big.tile([128, 2, LQ], BF16, tag="qtbf")

        def qt_mms(dc, nq, ps):
            for ic in range(IC1):
                nc.tensor.matmul(
                    ps[:],
                    lhsT=wqs[ic][:, dc * 128:(dc + 1) * 128],
                    rhs=d1T[:, ic, nq * 512:(nq + 1) * 512],
                    start=(ic == 0),
                    stop=(ic == IC1 - 1),
                )

        def qt_bias_sc(ps, dc, nq, on_act):
            o = qt_sb[:, dc, nq * 512:(nq + 1) * 512]
            if on_act:
                nc.scalar.activation(o, ps[:], AF.Identity,
                                     bias=bq_col[:, dc:dc + 1])
            else:
                nc.vector.tensor_scalar(o, ps[:], bq_col[:, dc:dc + 1], None, ADD)

        def qt_bias_rs(ps, dc, nq, on_act):
            o = qtbf[:, dc, nq * 512:(nq + 1) * 512]
            if on_act:
                nc.scalar.activation(o, ps[:], AF.Identity,
                                     bias=bqk_col[:, dc:dc + 1])
            else:
                nc.vector.tensor_scalar(o, ps[:], bqk_col[:, dc:dc + 1], None, ADD)

        def qt_unit(dc, nq, on_act=False):
            ps = ps_gp.tile([128, 512], F32, tag="ps_gp")
            qt_mms(dc, nq, ps)
            qt_bias_sc(ps, dc, nq, on_act)
            qt_bias_rs(ps, dc, nq, on_act)

        # ---------------- residual Q via xbar DMA transpose ------------------
        # qres3[qg][q_low, j, dc, c] = Q[qg*512 + j*128 + q_low, dc*128 + c]
        qres3 = [big.tile([128, 4, 2, 128], BF16, tag=f"qres{qg}",
                          name=f"qres{qg}")
                 for qg in range(4)]

        def qres_pe(qg, dc):
            pt = ps_tr.tile([128, 512], BF16, tag="ps_t", name=f"pt_q_{qg}_{dc}")
            for j in range(4):
                qb = qg * 4 + j
                nc.tensor.transpose(
                    pt[:, j * 128:(j + 1) * 128],
                    qtbf[:, dc, qb * 128:(qb + 1) * 128],
                    ident_bf[:],
                )
            nc.vector.tensor_copy(
                qres3[qg][:, :, dc, :],
                pt[:].rearrange("p (j c) -> p j c", c=128),
            )

        # ---------------- scores + exp --------------------------------------
        expT = [
            [big.tile([128, 2, 1024], FP8, tag=f"expT{kp}_{nh}",
                      name=f"expT{kp}_{nh}")
             for nh in range(2)]
            for kp in range(KP)
        ]

        def scores_unit(km, nh):
            ps = ps_sc.tile([128, 1024], F32, tag="ps_sc")
            for half in range(2):
                nq = nh * 2 + half
                nc.tensor.matmul(
                    ps[:, half * 512:(half + 1) * 512],
                    lhsT=kt_sb[:, :, km * 128:(km + 1) * 128],
                    rhs=qt_sb[:, :, nq * 512:(nq + 1) * 512],
                    perf_mode=PM_DR,
                    start=True,
                    stop=True,
                )
            nc.scalar.activation(
                expT[km // 2][nh][:, km % 2, :], ps[:], AF.Exp, scale=SCALE
            )

        # ---------------- context + residual + out DMA ----------------------
        out_c = [stage2.tile([128, 2 * D], F32, tag="outc", name=f"outc{c}")
                 for c in range(QB // 2)]
        ctxA = [big.tile([128, D + 1], BF16, tag=f"ctxA{i}", name=f"ctxA{i}")
                for i in range(8)]
        KP_A = 6  # h1 ctx kp-split: A = kp0-5 inside exp stream, B = kp6-7 after

        def ctx_mm(pc, qb, kp, start, stop):
            h, qq = qb // 8, qb % 8
            nc.tensor.matmul(
                pc,
                lhsT=expT[kp][h][:, :, qq * 128:(qq + 1) * 128],
                rhs=key2[kp][:],
                perf_mode=PM_DR,
                start=start,
                stop=stop,
            )

        def ctx_finish(pc, qb):
            rc = small.tile([128, 1], F32, tag="recip")
            nc.vector.reciprocal(rc[:], pc[:, D:D + 1])
            c = qb // 2
            osl = out_c[c][:, (qb % 2) * D:(qb % 2 + 1) * D]
            nc.vector.tensor_scalar(osl, pc[:, :D], rc[:], None, MULT)
            qg, j = qb // 4, qb % 4
            nc.gpsimd.tensor_add(
                osl.rearrange("p (a b) -> p a b", a=2),
                osl.rearrange("p (a b) -> p a b", a=2),
                qres3[qg][:, j, :, :],
            )
            if qb % 2 == 1:
                nc.sync.dma_start(
                    out=out[c * 256:(c + 1) * 256, :].rearrange(
                        "(t p) d -> p t d", p=128),
                    in_=out_c[c][:].rearrange("p (t d) -> p t d", d=D),
                )

        def ctx_unit_h0(qb):
            pc_full = ps_gp.tile([128, 512], F32, tag="ps_gp")
            pc = pc_full[:, :D + 1]
            for kp in range(KP):
                ctx_mm(pc, qb, kp, kp == 0, kp == KP - 1)
            ctx_finish(pc, qb)

        def ctx_h1_A(qb):
            pc_full = ps_gp.tile([128, 512], F32, tag="ps_gp")
            pc = pc_full[:, :D + 1]
            for kp in range(KP_A):
                ctx_mm(pc, qb, kp, kp == 0, kp == KP_A - 1)
            nc.vector.tensor_copy(ctxA[qb - 8][:], pc)

        def ctx_h1_B(qb):
            pc_full = ps_sc.tile([128, 512], F32, tag="ps_sc")
            pc = pc_full[:, :D + 1]
            for kp in range(KP_A, KP):
                ctx_mm(pc, qb, kp, kp == KP_A, kp == KP - 1)
            nc.vector.tensor_tensor(pc, pc, ctxA[qb - 8][:], ADD)
            ctx_finish(pc, qb)

        # ================= emission schedule ================================
        def units(fn, idxs):
            return [lambda i=i: fn(*i) if isinstance(i, tuple) else fn(i)
                    for i in idxs]

        def interleave(a, b, ratio):
            a = list(a)
            b = list(b)
            ia = ib = 0
            credit = 0.0
            while ia < len(a) or ib < len(b):
                if ia < len(a):
                    a[ia]()
                    ia += 1
                credit += ratio
                while credit >= 1.0 and ib < len(b):
                    b[ib]()
                    ib += 1
                    credit -= 1.0
            while ib < len(b):
                b[ib]()
                ib += 1

        # --- phase 1a: d1 g0 -> PE transposes -> QT nq0 (d1 lands first) ---
        d1_cast(0, on_act=True)
        for ic in range(IC1):
            d1T_pe(0, ic)
        for dc in range(2):
            qt_unit(dc, 0, on_act=True)

        # --- phase 1b: d2 h0 -> PE transposes -> kt-h0 ---
        d2_cast(0)
        for ic in range(IC2):
            for h4 in range(2):
                d2T_pe(0, ic, h4)
        for nk in range(2):
            for dc in range(2):
                kt_unit(dc, nk)

        # --- phase 1c: d1 g1 per-ic pipelined -> QT nq1 (gates exp#0) ---
        ps_nq1 = []
        for dc in range(2):
            ps = ps_gp.tile([128, 512], F32, tag="ps_gp", name=f"qtps1_{dc}")
            for ic in range(IC1):
                if dc == 0:
                    d1_cast_ic(1, ic)
                    d1T_pe(1, ic)
                nc.tensor.matmul(
                    ps[:],
                    lhsT=wqs[ic][:, dc * 128:(dc + 1) * 128],
                    rhs=d1T[:, ic, 512:1024],
                    start=(ic == 0),
                    stop=(ic == IC1 - 1),
                )
            ps_nq1.append(ps)
        for dc in range(2):
            qt_bias_sc(ps_nq1[dc], dc, 1, on_act=True)
        for dc in range(2):
            qt_bias_rs(ps_nq1[dc], dc, 1, on_act=False)

        # residual transposes for q-half 0 (PE)
        for qg in range(2):
            for dc in range(2):
                qres_pe(qg, dc)

        # --- phase 2: scores-h0 interleaved with late loads + DMA T ---
        filler = (
            [lambda: d2_cast(1)]
            + units(d2T_pe, [(1, ic, h4) for ic in range(IC2)
                             for h4 in range(2)])
            + units(kt_unit, [(dc, nk) for nk in range(2, 4)
                              for dc in range(2)])
            + [lambda: d1_cast(2)]
            + units(d1T_pe, [(2, ic) for ic in range(IC1)])
            + units(qt_unit, [(0, 2), (1, 2)])
            + [lambda: d1_cast(3)]
            + units(d1T_pe, [(3, ic) for ic in range(IC1)])
            + units(qt_unit, [(0, 3), (1, 3)])
            + units(qres_pe, [(qg, dc) for qg in range(2, 4)
                              for dc in range(2)])
            + units(key_pair, list(range(8)))
        )
        scores_h0 = units(scores_unit, [(km, 0) for km in range(KB)])
        interleave(scores_h0, filler, len(filler) / len(scores_h0))

        # --- phase 3: scores-h1 with ctx-h0 and ctx-h1-A ---
        # ctx_h1_A reads exps km0-11 of h1, so A units may only be emitted
        # after scores-h1 km11 (Tile orders by emission-time dependencies)
        sc_h1_a = units(scores_unit, [(km, 1) for km in range(12)])
        sc_h1_b = units(scores_unit, [(km, 1) for km in range(12, KB)])
        ctx0 = units(ctx_unit_h0, list(range(0, 8)))
        ctxa = units(ctx_h1_A, list(range(8, 16)))
        interleave(sc_h1_a, ctx0, len(ctx0) / len(sc_h1_a))
        interleave(sc_h1_b, ctxa, len(ctxa) / len(sc_h1_b))

        # --- phase 4: ctx-h1-B tail ---
        for qb in range(8, 16):
            ctx_h1_B(qb)

    nc.compile()
    return nc


_NC = None
_last_in_maps = None


def _get_nc():
    global _NC
    if _NC is None:
        _NC = _build()
    return _NC


def make_wpack(Wq, Wk, bq, bk):
    Wq = np.asarray(Wq, dtype=np.float32)
    Wk = np.asarray(Wk, dtype=np.float32)
    bq = np.asarray(bq, dtype=np.float32)
    bk = np.asarray(bk, dtype=np.float32)
    wpack = np.empty((128, 2564), np.float32)
    for c in range(8):
        wpack[:, c * 256:(c + 1) * 256] = Wq[c * 128:(c + 1) * 128, :]
    for c in range(2):
        wpack[:, 2048 + c * 256:2048 + (c + 1) * 256] = Wk[c * 128:(c + 1) * 128, :]
    for c in range(2):
        wpack[:, 2560 + c] = bq[c * 128:(c + 1) * 128]
        wpack[:, 2562 + c] = bk[c * 128:(c + 1) * 128]
    return np.ascontiguousarray(wpack)


def kernel(data1, data2, Wq, bq, Wk, bk):
    global _last_in_maps
    data1 = np.asarray(data1, dtype=np.float32)
    data2 = np.asarray(data2, dtype=np.float32)
    wpack = make_wpack(Wq, Wk, bq, bk)

    nc = _get_nc()
    in_maps = [
        {
            "data1": np.ascontiguousarray(data1[b]),
            "data2": np.ascontiguousarray(data2[b]),
            "wpack": wpack,
        }
        for b in range(B)
    ]
    _last_in_maps = in_maps
    res = run_bass_kernel_spmd(nc, in_maps, core_ids=list(range(N_CORES)))
    return np.stack([res.results[i]["out"] for i in range(B)], axis=0)
